# revision 35
# baseline (speedup 1.0000x reference)
"""K-means nearest-centroid assignment on Trainium2, data-parallel across 8 cores.

Reference computes argmin_k ||x_n - c_k||^2. Since ||x_n||^2 is constant per
point, argmin_k d2 == argmax_k (x_n . c_k - 0.5*||c_k||^2). Each core gets
N/8 points (transposed on host so the contraction dim C lands on SBUF
partitions), the centroid table is replicated, and x.cT scores accumulate in
PSUM via PE matmuls.

Perf structure (PE-bound by design):
- Main matmuls run as float32r: 1 cycle/row on the PE when the moving dim is
  >=256, vs 4 cycles/row for exact fp32. Measured assignment error vs the
  fp32 reference is ~35/131072 points.
- No bias matmuls on the PE: the ACT engine (otherwise idle) evacuates each
  psum tile to SBUF (GpSimd has no PSUM port), GpSimd adds the bias row
  (~4.6us; plain add is the one elementwise op Pool supports on TRN2), and
  the DVE runs MAX8 + FIND_INDEX8 (~4.5us). GpSimd and the DVE ping-pong as
  co-bottlenecks at ~4.6us/subtile, just above the PE's 3.95.
  Measured dead ends: tensor_tensor_reduce (would fuse bias-add + max on
  the DVE) compiles and simulates but dies at runtime; scalar_tensor_tensor
  / tensor_tensor_scan / comparison ALU ops are rejected on Pool by the
  ISA; splitting the bias-add or the bias between engines (DVE share, or a
  PE bf16 bias matmul) measured slower than this shape due to DVE FIFO
  head-blocking and PE cadence disruption.
- cT is DMA'd as 16 [128,512] tiles in matmul-consumption order and the
  first x supertile is prefetched before the constants, so the PE starts
  ~2 MB (not ~5 MB) into the DMA stream.
"""

import sys

sys.path.insert(0, "/opt/trn_rl_repo")

import numpy as np
import ml_dtypes

import concourse.bass as bass
import concourse.bacc as bacc
import concourse.mybir as mybir
from concourse.tile import TileContext

N, C, K = 131072, 512, 2048
NCORES = 8
P = 128
KT = 512              # psum bank width in fp32 / matmul max moving dim
NKT = K // KT         # 4 K-tiles
NCC = C // P          # 4 contraction chunks
ST = 512              # points per supertile (xT DMA free dim)
NBIAS = 4             # bf16 pieces in the K-tile-0 bias decomposition

F32 = mybir.dt.float32
U32 = mybir.dt.uint32
MM_DT = mybir.dt.float32r
NEG_INF = -3.0e38
ALU = mybir.AluOpType


def build_nc(nloc, mm_dt=MM_DT):
    """One SPMD program: nloc points per core, full K centroids."""
    nsuper = nloc // ST
    nsub = ST // P

    nc = bacc.Bacc(None, target_bir_lowering=False)
    xT = nc.declare_dram_parameter("xT", [C, nloc], mm_dt, isOutput=False)
    cT = nc.declare_dram_parameter("cT", [C, K], mm_dt, isOutput=False)
    # bias[p, k] = -0.5*||c_k||^2, replicated across the 128 partitions.
    bias = nc.declare_dram_parameter("bias", [P, K], F32, isOutput=False)
    out = nc.declare_dram_parameter("out", [nloc], U32, isOutput=True)

    with TileContext(nc) as tc:
        with (
            tc.tile_pool(name="const", bufs=1) as const_pool,
            tc.tile_pool(name="xin", bufs=4) as xin_pool,
            tc.tile_pool(name="stt", bufs=6) as stt_pool,
            tc.tile_pool(name="res", bufs=8) as res_pool,
            tc.tile_pool(name="psum", bufs=2, space="PSUM") as psum_pool,
        ):
            def load_x(st):
                tiles = []
                for c in range(NCC):
                    t = xin_pool.tile([P, ST], mm_dt, tag=f"x{c}")
                    nc.sync.dma_start(
                        out=t[:], in_=xT[c * P:(c + 1) * P, st * ST:(st + 1) * ST]
                    )
                    tiles.append(t)
                return tiles

            # Prefetch the first x supertile before the big constants so the
            # PE can start as soon as x(st=0) and cT[c=0] have landed.
            x0_tiles = load_x(0)
            cT_tiles = []
            for c in range(NCC):
                row = []
                for j in range(NKT):
                    t = const_pool.tile([P, KT], mm_dt, tag=f"cT{c}_{j}")
                    nc.sync.dma_start(
                        out=t[:], in_=cT[c * P:(c + 1) * P, j * KT:(j + 1) * KT]
                    )
                    row.append(t)
                cT_tiles.append(row)
            bias_t = const_pool.tile([P, K], F32, tag="bias")
            nc.sync.dma_start(out=bias_t[:], in_=bias[:, :])

            for st in range(nsuper):
                n0 = st * ST
                x_tiles = x0_tiles if st == 0 else load_x(st)
                for s in range(nsub):
                    ps = psum_pool.tile([P, K], F32, tag="ps")
                    for c in range(NCC):
                        for j in range(NKT):
                            nc.tensor.matmul(
                                ps[:, j * KT:(j + 1) * KT],
                                lhsT=x_tiles[c][:, s * P:(s + 1) * P],
                                rhs=cT_tiles[c][j][:],
                                start=(c == 0),
                                stop=(c == NCC - 1),
                            )
                    # ACT evacuates psum (GpSimd has no PSUM port).
                    s_sb = stt_pool.tile([P, K], F32, tag="ssb")
                    nc.scalar.copy(out=s_sb[:], in_=ps[:])
                    # GpSimd: biased = scores + bias.
                    biased = stt_pool.tile([P, K], F32, tag="biased")
                    nc.gpsimd.tensor_tensor(
                        out=biased[:], in0=s_sb[:], in1=bias_t[:], op=ALU.add
                    )
                    # DVE: row max, then index of its first occurrence.
                    m8 = res_pool.tile([P, 8], F32, tag="m8")
                    i8 = res_pool.tile([P, 8], U32, tag="i8")
                    nc.vector.max(m8[:], biased[:])
                    nc.vector.max_index(i8[:], m8[:], biased[:])
                    nc.sync.dma_start(
                        out=out[n0 + s * P:n0 + (s + 1) * P], in_=i8[:, 0:1]
                    )
    nc.finalize()
    return nc


def make_in_maps(inp, centroids, nloc=None, ncores=NCORES):
    inp = np.asarray(inp, dtype=np.float32)
    centroids = np.asarray(centroids, dtype=np.float32)
    if nloc is None:
        nloc = inp.shape[0] // ncores
    cT = np.ascontiguousarray(centroids.T)
    c2 = np.sum(centroids.astype(np.float64) ** 2, axis=1)
    bias_row = (-0.5 * c2).astype(np.float32)
    bias = np.ascontiguousarray(np.broadcast_to(bias_row[None, :], (P, K)))
    in_maps = []
    for i in range(ncores):
        xl = inp[i * nloc:(i + 1) * nloc]
        in_maps.append(
            {
                "xT": np.ascontiguousarray(xl.T),
                "cT": cT,
                "bias": bias,
            }
        )
    return in_maps


def kernel(inp, centroids):
    from concourse.bass_utils import run_bass_kernel_spmd

    nloc = N // NCORES
    nc = build_nc(nloc)
    in_maps = make_in_maps(inp, centroids, nloc=nloc)
    res = run_bass_kernel_spmd(nc, in_maps, core_ids=list(range(NCORES)))
    parts = [res.results[i]["out"].reshape(-1) for i in range(NCORES)]
    return np.concatenate(parts).astype(np.int32)
